# revision 33
# baseline (speedup 1.0000x reference)
"""Low-rank linear kernel for Trainium2 (8 NeuronCores, data-parallel).

Computes out = input @ (A @ B).T with A:[4096,16], B:[16,4096],
input:[4,4096,4096] via the rank-16 factorization:
    t   = x @ B.T   (contract 4096, accumulated per 128-feature chunk)
    out = t @ A.T   (contract 16)

Input rows are sharded 8 ways across cores (2048 rows each); A,B are
replicated. The kernel is HBM-bandwidth-bound, so everything is built
around minimizing and saturating the two DMA streams:

- x and y live in HBM as bf16 (host casts f32<->bf16): 16 MB in + 16 MB
  out per core instead of 64 MB total for f32. Matmuls run bf16 with
  fp32 PSUM accumulation; end-to-end rel-err ~3.5e-3 vs the 2e-2 gate.
- x is uploaded BLOCK-TRANSPOSED (see make_global_inputs): each 128x128
  block of a row tile is stored features-on-partitions, so the rank-16
  contraction needs NO on-device transposes -- each x chunk is directly
  the stationary (lhsT) operand, with the 16-wide B.T chunk streaming.
  Only a tiny [128,16]->[16,128] t-transpose per row tile remains.
- Row tiles ride 2-per-DMA (2 MB transfers) to amortize DMA fixed cost;
  x loads go out on the SP HWDGE ring, y stores on the Activation ring.
- PSUM->SBUF output copies alternate between ACT and DVE.

Modeled steady state (TimelineSim): DMA engines ~96% busy; PE ~35%,
ACT/DVE ~45% each, all hidden under the DMA streams. Measured ~80 us
per pass per core (~400 GB/s effective) vs the 217 us f32 baseline.

kernel() caches the compiled sharded executable and the device-resident
zero output-init buffers, so repeat calls only upload x/A/B and run.
"""
import numpy as np
import concourse.bass as bass
from concourse import bacc
import concourse.mybir as mybir
import concourse.tile as tile
from concourse.bass_utils import run_bass_kernel_spmd
from concourse.masks import make_identity

F32 = mybir.dt.float32
F32R = mybir.dt.float32r
BF16 = mybir.dt.bfloat16

N_CORES = 8
ROWS_TOTAL = 4 * 4096
R = ROWS_TOTAL // N_CORES  # 2048 rows per core
D = 4096
RANK = 16
RT = 128                   # row tile
N_RT = R // RT             # 16 row tiles per core
KC = D // 128              # 32 contraction chunks
TG = 4                     # transpose group: chunks per PSUM bank
NG = KC // TG              # 8 groups per row tile
NJ = D // 512              # 8 output column chunks

_CACHE = {}


def _build(reps=1, py_unroll=False, dma_only=False):
    nc = bacc.Bacc("TRN2", debug=False, num_devices=N_CORES)
    # x and y live in HBM as bf16: the kernel is HBM-bandwidth-bound
    # (~358 GB/s/core), so halving both streams vs f32 nearly halves HW
    # time. The host casts f32->bf16 on upload and bf16->f32 on return;
    # rel-err stays ~4e-3, well under the 2e-2 gate.
    #
    # x additionally arrives BLOCK-TRANSPOSED from the host (see
    # make_global_inputs): dram row RT*m+p, col 128*c+r holds
    # x_core[RT*m+r, 128*c+p]. Each 128x128 block of a row tile is thus
    # stored features-on-partitions, so the main loop needs NO on-device
    # transposes: x_t[:, 128c:128c+128] is directly the lhsT (stationary)
    # operand of the rank-16 contraction. This removes 32 PE transposes,
    # 32 ldweights and 8 PSUM->SBUF copies per row tile vs the previous
    # version, leaving the DMA streams as the only near-saturated resource.
    x = nc.dram_tensor("x", [R, D], BF16, kind="ExternalInput")
    a = nc.dram_tensor("a", [D, RANK], F32, kind="ExternalInput")
    b = nc.dram_tensor("b", [RANK, D], F32, kind="ExternalInput")
    y = nc.dram_tensor("y", [R, D], BF16, kind="ExternalOutput")

    with tile.TileContext(nc) as tc:
        with (
            tc.tile_pool(name="const", bufs=1) as const,
            tc.tile_pool(name="xin", bufs=4) as xin,
            tc.tile_pool(name="t_ps", bufs=2, space="PSUM") as t_ps_pool,
            tc.tile_pool(name="t_sb", bufs=2) as t_sb_pool,
            tc.tile_pool(name="tT_ps", bufs=2, space="PSUM") as tT_ps_pool,
            tc.tile_pool(name="tT_sb", bufs=2) as tT_sb_pool,
            tc.tile_pool(name="o_ps", bufs=4, space="PSUM") as o_ps_pool,
            tc.tile_pool(name="o_sb", bufs=3) as o_sb_pool,
        ):
            # ---- constants / preprocessing (once per core) ----
            ident = const.tile([128, 128], F32)
            make_identity(nc, ident[:])
            # bf16 identity for the tiny per-tile t transpose
            ident_h = const.tile([128, 128], BF16)
            nc.vector.tensor_copy(ident_h[:], ident[:])

            # B natural [16, D]
            b_nat = const.tile([RANK, D], F32)
            nc.sync.dma_start(b_nat[:], b[:])
            # bt [128, 16*KC] bf16: chunk c at cols 16c:16c+16 = B[:,128c:128c+128].T
            bt = const.tile([128, RANK * KC], BF16)
            for g in range(NG):
                ps = o_ps_pool.tile([128, RANK * TG], F32, name="pre_b", tag="o")
                for s in range(TG):
                    c = g * TG + s
                    nc.tensor.transpose(
                        ps[:, RANK * s:RANK * (s + 1)],
                        b_nat[:, 128 * c:128 * (c + 1)],
                        ident[:RANK, :RANK],
                    )
                nc.scalar.copy(bt[:, RANK * TG * g:RANK * TG * (g + 1)], ps[:])

            # A natural, packed [128, 16*KC]: chunk c at cols 16c = A[128c:128c+128,:]
            a_nat = const.tile([128, RANK * KC], F32)
            nc.sync.dma_start(
                a_nat[:].rearrange("p (c r) -> p c r", r=RANK),
                a[:].rearrange("(c p) r -> p c r", p=128),
            )
            # at [16, D] bf16 = A.T
            at = const.tile([RANK, D], BF16)
            for g in range(NG):
                ps = o_ps_pool.tile([RANK, 512], F32, name="pre_a", tag="o")
                for s in range(TG):
                    c = g * TG + s
                    nc.tensor.transpose(
                        ps[:, 128 * s:128 * (s + 1)],
                        a_nat[:, RANK * c:RANK * (c + 1)],
                        ident[:],
                    )
                nc.scalar.copy(at[:, 512 * g:512 * (g + 1)], ps[:])

            # ---- main loop over row tiles ----
            # reps>1 wraps the identical 16-tile body in a HW loop (for
            # slope timing); body does not use the loop variable.
            import contextlib
            hints = ((mybir.EngineType.Activation, mybir.EngineType.SP)
                     if dma_only else
                     (mybir.EngineType.PE, mybir.EngineType.DVE,
                      mybir.EngineType.Activation, mybir.EngineType.SP))
            loop = (tc.For_i(0, reps, 1, hint_engines=hints)
                    if reps > 1 and not py_unroll else contextlib.nullcontext())
            # SUP row tiles ride in each DMA (2 MB transfers: per partition,
            # SUP contiguous 8 KB segments) to amortize per-DMA fixed cost.
            SUP = 2
            NS = N_RT // SUP
            n_iters = NS * (reps if py_unroll else 1)
            # DMA-roofline probe: same transfers, no compute. Stores read a
            # single zeroed buffer so they have no producer dependencies.
            o_const = None
            if dma_only:
                o_const = const.tile([128, SUP * D], BF16, name="o_const")
                nc.vector.memset(o_const[:], 0.0)
            with loop:
              for it in range(n_iters):
                k = it % NS
                x2 = xin.tile([128, SUP * D], BF16)
                nc.sync.dma_start(
                    x2[:].rearrange("p (t d) -> p t d", d=D),
                    x[RT * SUP * k:RT * SUP * (k + 1), :].rearrange(
                        "(t p) d -> p t d", p=128),
                )
                if dma_only:
                    nc.scalar.dma_start(
                        y[RT * SUP * k:RT * SUP * (k + 1), :].rearrange(
                            "(t p) d -> p t d", p=128),
                        o_const[:].rearrange("p (t d) -> p t d", d=D),
                    )
                    continue
                o_sb = o_sb_pool.tile([128, SUP * D], BF16)
                for u in range(SUP):
                    x_t = x2[:, u * D:(u + 1) * D]
                    # t_nat [128 rows, 16] = x_tile @ B.T, accumulated over
                    # the 32 feature chunks. The x chunk (already transposed
                    # in SBUF) is the stationary operand; the 16-wide B.T
                    # chunk streams, so each matmul is ldweights-paced.
                    t_ps = t_ps_pool.tile([RT, RANK], F32)
                    for c in range(KC):
                        nc.tensor.matmul(
                            t_ps[:],
                            x_t[:, 128 * c:128 * (c + 1)],
                            bt[:, RANK * c:RANK * (c + 1)],
                            start=(c == 0),
                            stop=(c == KC - 1),
                        )
                    t_sb = t_sb_pool.tile([RT, RANK], BF16)
                    nc.scalar.copy(t_sb[:], t_ps[:])
                    # tiny per-tile transpose: t [128,16] -> tT [16,128]
                    tT_ps = tT_ps_pool.tile([RANK, RT], BF16)
                    nc.tensor.transpose(tT_ps[:], t_sb[:], ident_h[:])
                    tT_sb = tT_sb_pool.tile([RANK, RT], BF16)
                    nc.vector.tensor_copy(tT_sb[:], tT_ps[:])

                    for j in range(NJ):
                        o_ps = o_ps_pool.tile([RT, 512], F32, tag="o")
                        nc.tensor.matmul(
                            o_ps[:], tT_sb[:], at[:, 512 * j:512 * (j + 1)],
                            start=True, stop=True,
                        )
                        dst = o_sb[:, u * D + 512 * j:u * D + 512 * (j + 1)]
                        if j % 2 == 0:
                            nc.vector.tensor_copy(dst, o_ps[:])
                        else:
                            nc.scalar.copy(dst, o_ps[:])
                # Activation HWDGE queue: separate ring from the SP input
                # queue so in/out transfers can overlap on hardware.
                nc.scalar.dma_start(
                    y[RT * SUP * k:RT * SUP * (k + 1), :].rearrange(
                        "(t p) d -> p t d", p=128),
                    o_sb[:].rearrange("p (t d) -> p t d", d=D),
                )

    nc.compile()
    return nc


def get_nc(reps=1):
    key = ("nc", reps)
    if key not in _CACHE:
        _CACHE[key] = _build(reps)
    return _CACHE[key]


def get_nc_dma(reps=1):
    """DMA-only roofline probe build (diagnostic, not used by kernel())."""
    key = ("nc_dma", reps)
    if key not in _CACHE:
        _CACHE[key] = _build(reps, dma_only=True)
    return _CACHE[key]


def get_nc_sim(reps=1):
    """Python-unrolled build (no For_i) — TimelineSim can't resolve the
    register branch of the hardware loop, so simulate an unrolled body."""
    key = ("nc_sim", reps)
    if key not in _CACHE:
        _CACHE[key] = _build(reps, py_unroll=True)
    return _CACHE[key]


def make_in_maps(input, A, B):
    xf = _host_x(input)
    A = np.ascontiguousarray(np.asarray(A, dtype=np.float32))
    B = np.ascontiguousarray(np.asarray(B, dtype=np.float32))
    return [
        {"x": xf[R * i:R * (i + 1)], "a": A, "b": B}
        for i in range(N_CORES)
    ]


def _get_exec(nc):
    """Build (once) a reusable jitted sharded executable for nc, plus the
    device-resident zero output-init buffers."""
    key = ("exec", id(nc))
    if key in _CACHE:
        return _CACHE[key]
    import jax
    from jax.sharding import Mesh, PartitionSpec, NamedSharding
    from jax.experimental.shard_map import shard_map
    from concourse import bass2jax

    bass2jax.install_neuronx_cc_hook()
    partition_name = nc.partition_id_tensor.name if nc.partition_id_tensor else None
    in_names, out_names, out_avals, zero_outs = [], [], [], []
    for alloc in nc.m.functions[0].allocations:
        if not isinstance(alloc, mybir.MemoryLocationSet):
            continue
        name = alloc.memorylocations[0].name
        if alloc.kind == "ExternalInput":
            if name != partition_name:
                in_names.append(name)
        elif alloc.kind == "ExternalOutput":
            shape = tuple(alloc.tensor_shape)
            dtype = mybir.dt.np(alloc.dtype)
            out_names.append(name)
            out_avals.append(jax.core.ShapedArray(shape, dtype))
            zero_outs.append(np.zeros(shape, dtype))
    n_params = len(in_names)
    all_in_names = tuple(
        in_names + out_names + ([partition_name] if partition_name else [])
    )

    def _body(*args):
        operands = list(args)
        if partition_name is not None:
            operands.append(bass2jax.partition_id_tensor())
        outs = bass2jax._bass_exec_p.bind(
            *operands,
            out_avals=tuple(out_avals),
            in_names=all_in_names,
            out_names=tuple(out_names),
            lowering_input_output_aliases=(),
            sim_require_finite=True,
            sim_require_nnan=True,
            nc=nc,
        )
        return tuple(outs)

    devices = jax.devices()[:N_CORES]
    mesh = Mesh(np.asarray(devices), ("core",))
    n_outs = len(out_avals)
    fn = jax.jit(
        shard_map(
            _body, mesh=mesh,
            in_specs=(PartitionSpec("core"),) * (n_params + n_outs),
            out_specs=(PartitionSpec("core"),) * n_outs,
            check_rep=False,
        ),
        keep_unused=True,
    )
    sharding = NamedSharding(mesh, PartitionSpec("core"))
    dev_zeros = [
        jax.device_put(
            np.zeros((N_CORES * z.shape[0], *z.shape[1:]), z.dtype), sharding
        )
        for z in zero_outs
    ]
    res = (fn, in_names, out_avals, dev_zeros, sharding)
    _CACHE[key] = res
    return res


def _host_x(input):
    """bf16 cast + block-transpose of x to the kernel's dram layout:
    dram[128*t + p, 128*c + r] = x[128*t + r, 128*c + p]."""
    import ml_dtypes
    xf = np.asarray(input, dtype=np.float32).reshape(ROWS_TOTAL, D)
    xh = xf.astype(ml_dtypes.bfloat16)
    xb = xh.reshape(ROWS_TOTAL // RT, RT, KC, 128).transpose(0, 3, 2, 1)
    return np.ascontiguousarray(xb).reshape(ROWS_TOTAL, D)


def make_global_inputs(input, A, B):
    """Full-mesh host arrays keyed by dram tensor name (concat over cores)."""
    af = np.tile(np.ascontiguousarray(np.asarray(A, dtype=np.float32)), (N_CORES, 1))
    bf = np.tile(np.ascontiguousarray(np.asarray(B, dtype=np.float32)), (N_CORES, 1))
    return {"x": _host_x(input), "a": af, "b": bf}


def _run_cached(nc, input, A, B):
    import jax
    fn, in_names, out_avals, dev_zeros, sharding = _get_exec(nc)
    global_in = make_global_inputs(input, A, B)
    args = [jax.device_put(global_in[name], sharding) for name in in_names]
    out_arrs = fn(*args, *dev_zeros)
    y = np.asarray(out_arrs[0]).astype(np.float32)  # single output "y", bf16 -> f32
    return y.reshape(np.asarray(input).shape)


def kernel(input, A, B):
    nc = get_nc()
    try:
        return _run_cached(nc, input, A, B)
    except Exception:
        in_maps = make_in_maps(input, A, B)
        res = run_bass_kernel_spmd(nc, in_maps, core_ids=list(range(N_CORES)))
        out = np.concatenate([r["y"] for r in res.results], axis=0).astype(np.float32)
        return out.reshape(np.asarray(input).shape)

